# revision 27
# baseline (speedup 1.0000x reference)
"""Trainium2 Bass kernel for nn_DDPMITERVAEQueryEncoder.

Computation: pooled-VAE encoder (embedding gather + 2-layer MLP) producing a
latent z per batch row, then a 200-step DDPM reverse loop on [B, 64] states
conditioned on z.

Strategy (8 NeuronCores, data-parallel over batch):
  - batch B=8192 sharded 1024 rows/core; embedding table replicated in DRAM,
    gathered on-device via indirect DMA (the memory-bound part).
  - diffusion state kept on-chip as a rescaled variable
        w_t = sqrt(1/acp_t) * x_t - sqrt(1/acp_t - 1) * b_out
    which folds every per-step affine constant into either a compile-time
    immediate, a per-partition bias table, or the host-precomputed noise slab,
    so each step is 4 matmuls + 1 activation + a handful of fused vector ops.
  - the PRNG noise (jax threefry, input-independent) is reproduced bit-exactly
    with jax on host CPU and streamed from DRAM.

Layouts ("stacked" [128, 512] tile, per core): partition q = s*64 + d,
column n; local row = s*512 + n, feature dim d.
"""

import os
import numpy as np

D = 64
T = 200
B = 8192
L = 100
N_ITEMS = 100000
NCORES = 8
BS = B // NCORES            # 1024 rows per core
SUB = 256                   # free-dim of each of the 2 pipelined subtiles
NJ = BS // 128              # 8 row-groups of 128 rows (row = 128*j + p)
NQ = 4                      # gather: table split into 4 quarters (int16 reach)
QROWS = N_ITEMS // NQ       # 25000 rows per quarter (+1 appended zero row)
K_LADDER = (48, 64, 100)    # per-(row, quarter) slot budgets

_cache: dict = {}


# ----------------------------------------------------------------------------
# host-side constant prep
# ----------------------------------------------------------------------------

def _jax_cpu():
    import jax
    return jax, jax.devices("cpu")[0]


def _schedule_np():
    """Faithful fp32 replica of the reference _schedule(), via jax cpu."""
    if "sched" in _cache:
        return _cache["sched"]
    jax, cpu = _jax_cpu()
    import jax.numpy as jnp
    with jax.default_device(cpu):
        scale = 1000.0 / T
        betas = jnp.linspace(scale * 1e-4, scale * 0.02, T, dtype=jnp.float32)
        alphas = 1.0 - betas
        acp = jnp.cumprod(alphas)
        acp_prev = jnp.concatenate([jnp.ones((1,), jnp.float32), acp[:-1]])
        sqrt_recip = jnp.sqrt(1.0 / acp)
        sqrt_recipm1 = jnp.sqrt(1.0 / acp - 1.0)
        post_var = betas * (1.0 - acp_prev) / (1.0 - acp)
        post_logvar = jnp.log(jnp.concatenate([post_var[1:2], post_var[1:]]))
        coef1 = betas * jnp.sqrt(acp_prev) / (1.0 - acp)
        coef2 = (1.0 - acp_prev) * jnp.sqrt(alphas) / (1.0 - acp)
        out = tuple(np.asarray(x, np.float64) for x in
                    (sqrt_recip, sqrt_recipm1, post_logvar, coef1, coef2))
    _cache["sched"] = out
    return out


def _host_noise():
    """x_init and per-step noise, bit-exact with the reference PRNG."""
    if "noise" in _cache:
        return _cache["noise"]
    jax, cpu = _jax_cpu()
    import jax.numpy as jnp
    with jax.default_device(cpu):
        key2 = jax.random.key(2)
        f = jax.jit(lambda t: jax.random.normal(
            jax.random.fold_in(key2, t), (B, D), jnp.float32))
        noise = np.empty((T, B, D), np.float32)
        for t in range(T):
            noise[t] = np.asarray(f(jnp.int32(t)))
        x_init = np.asarray(jax.random.normal(jax.random.key(1), (B, D),
                                              jnp.float32))
    _cache["noise"] = (x_init, noise)
    return _cache["noise"]


def _derived_consts():
    if "consts" in _cache:
        return _cache["consts"]
    sr, srm1, post_logvar, c1, c2 = _schedule_np()
    sig = np.exp(0.5 * post_logvar)
    sig[0] = 0.0
    sr_prev = np.empty(T); srm1_prev = np.empty(T)
    sr_prev[1:] = sr[:-1]; sr_prev[0] = 1.0
    srm1_prev[1:] = srm1[:-1]; srm1_prev[0] = 0.0
    a_t = sr_prev * c1
    b_t = sr_prev * c2 / sr
    noise_coef = sr_prev * sig
    # state rescaling: device state S_t = beta_t * w_t, chosen so the
    # "tmp = b_t*w + sn" op becomes a plain add: beta_{t-1} = beta_t / b_t,
    # anchored at beta_0 = 1 (t=0 is exempt since b_0 = 0).
    beta = np.ones(T)
    beta[1:] = np.exp(np.cumsum(np.log(b_t[1:])))
    beta_prev = np.empty(T)              # beta_{t-1}; beta_{-1} := 1
    beta_prev[0] = 1.0
    beta_prev[1:] = beta[:-1]
    stt_scalar = a_t * beta_prev / beta  # = a_t/b_t for t>=1, a_0 at t=0
    out = dict(sr=sr, srm1=srm1, a_t=a_t, b_t=b_t, noise_coef=noise_coef,
               srm1_prev=srm1_prev, beta=beta, beta_prev=beta_prev,
               stt_scalar=stt_scalar)
    _cache["consts"] = out
    return out


def _stack(arr):
    """[1024, 64] -> stacked [128, 512]."""
    return np.ascontiguousarray(
        arr.reshape(2, 512, 64).transpose(0, 2, 1).reshape(128, 512))


def _unstack(tile):
    """stacked [128, 512] -> [1024, 64]."""
    return tile.reshape(2, 64, 512).transpose(0, 2, 1).reshape(1024, 64)


def _blockdiag(w):
    out = np.zeros((128, 128), np.float32)
    out[:64, :64] = w
    out[64:, 64:] = w
    return out


# ----------------------------------------------------------------------------
# bass program (built & compiled once per process)
# ----------------------------------------------------------------------------

def _build_program(sn_dtype_np, K):
    import concourse.bass as bass
    import concourse.bacc as bacc
    import concourse.mybir as mybir
    import concourse.tile as tile

    f32 = mybir.dt.float32
    i16 = mybir.dt.int16
    sn_dt = mybir.dt.from_np(np.dtype(sn_dtype_np))
    Alu = mybir.AluOpType
    Act = mybir.ActivationFunctionType

    con = _derived_consts()
    sr, srm1 = con["sr"], con["srm1"]
    beta, stt_scalar = con["beta"], con["stt_scalar"]

    nc = bacc.Bacc("TRN2", target_bir_lowering=False, debug=False,
                   num_devices=NCORES)

    NCALL = (NJ // 2) * NQ          # j-pair x quarter gather calls
    NIDX = 128 * 2 * K              # indices per call
    # DRAM I/O (per core)
    idx_d = nc.dram_tensor("idx", [128, NCALL * (NIDX // 16)], i16,
                           kind="ExternalInput").ap()
    embq_d = [nc.dram_tensor(f"embq{q}", [QROWS + 1, D], f32,
                             kind="ExternalInput").ap() for q in range(NQ)]
    rsq_d = nc.dram_tensor("rsq", [128, 512], f32, kind="ExternalInput").ap()
    we1_d = nc.dram_tensor("wenc1", [64, 256], f32, kind="ExternalInput").ap()
    w2a_d = nc.dram_tensor("w2a", [128, 64], f32, kind="ExternalInput").ap()
    w2b_d = nc.dram_tensor("w2b", [128, 64], f32, kind="ExternalInput").ap()
    b1_d = nc.dram_tensor("b1", [128, 2], f32, kind="ExternalInput").ap()
    b2_d = nc.dram_tensor("b2", [64, 1], f32, kind="ExternalInput").ap()
    win_d = nc.dram_tensor("win_bd", [128, 128], f32, kind="ExternalInput").ap()
    eye_d = nc.dram_tensor("eye128", [128, 128], f32, kind="ExternalInput").ap()
    tb_d = nc.dram_tensor("tbias", [128, T], f32, kind="ExternalInput").ap()
    w0_d = nc.dram_tensor("w0", [128, 512], f32, kind="ExternalInput").ap()
    # per-step slab: cols 0:512 noise, 512:640 scaled-W_c, 640:768 scaled-W_out
    sn_d = nc.dram_tensor("sn", [T, 128, 768], sn_dt, kind="ExternalInput").ap()
    out_d = nc.dram_tensor("out", [128, 512], f32, kind="ExternalOutput").ap()

    with tile.TileContext(nc) as tc:
        import contextlib
        with (
            tc.tile_pool(name="const", bufs=1) as cpool,
            tc.tile_pool(name="work", bufs=2) as wpool,
            tc.tile_pool(name="state", bufs=2) as spool,
            tc.tile_pool(name="noise", bufs=4) as npool,
            contextlib.ExitStack() as phase_ctx,
        ):
            gpool = phase_ctx.enter_context(tc.tile_pool(name="gather", bufs=2))
            ppool = phase_ctx.enter_context(
                tc.tile_pool(name="psum_enc", bufs=2, space="PSUM"))
            # ---- load constants ----
            rsq_t = cpool.tile([128, 512], f32, tag="rsq")
            nc.sync.dma_start(out=rsq_t[:], in_=rsq_d[:])
            we1_t = cpool.tile([64, 256], f32, tag="we1")
            nc.sync.dma_start(out=we1_t[:], in_=we1_d[:])
            w2a_t = cpool.tile([128, 64], f32, tag="w2a")
            nc.sync.dma_start(out=w2a_t[:], in_=w2a_d[:])
            w2b_t = cpool.tile([128, 64], f32, tag="w2b")
            nc.sync.dma_start(out=w2b_t[:], in_=w2b_d[:])
            b1_t = cpool.tile([128, 2], f32, tag="b1")
            nc.sync.dma_start(out=b1_t[:], in_=b1_d[:])
            b2_t = cpool.tile([64, 1], f32, tag="b2")
            nc.sync.dma_start(out=b2_t[:], in_=b2_d[:])
            win_t = cpool.tile([128, 128], f32, tag="win")
            nc.sync.dma_start(out=win_t[:], in_=win_d[:])
            eye_t = cpool.tile([128, 128], f32, tag="eye")
            nc.sync.dma_start(out=eye_t[:], in_=eye_d[:])
            tb_t = cpool.tile([128, T], f32, tag="tb")
            nc.sync.dma_start(out=tb_t[:], in_=tb_d[:])

            # ---- phase 1: gather + pool ----
            # call (jp, q): rows 256*jp..256*jp+255, table quarter q.
            # entry i -> partition i%128, slot i//128 = jj*K + k
            idxg_t = gpool.tile([128, NCALL * (NIDX // 16)], i16, tag="idxg",
                                bufs=1)
            nc.sync.dma_start(out=idxg_t[:], in_=idx_d[:])
            redq = [cpool.tile([128, 512], f32, tag=f"redq{q}", name=f"redq{q}")
                    for q in range(NQ)]
            for jp in range(NJ // 2):
                for q in range(NQ):
                    ci = jp * NQ + q
                    g = gpool.tile([128, 2 * K * D], f32, tag="g")
                    nc.gpsimd.dma_gather(
                        out_ap=g[:].rearrange("p (s d) -> p s d", d=D),
                        in_ap=embq_d[q][:],
                        idxs_ap=idxg_t[:, ci * (NIDX // 16):(ci + 1) * (NIDX // 16)],
                        num_idxs=NIDX, num_idxs_reg=NIDX, elem_size=D,
                        single_packet=False)
                    gv = g[:].rearrange("p (jj k d) -> p jj d k", jj=2, k=K)
                    nc.vector.tensor_reduce(
                        out=redq[q][:, jp * 128:(jp + 1) * 128]
                            .rearrange("p (jj d) -> p jj d", jj=2),
                        in_=gv, axis=mybir.AxisListType.X, op=Alu.add)
            pooled = cpool.tile([128, 512], f32, tag="pooled")
            nc.vector.tensor_tensor(out=redq[0][:], in0=redq[0][:],
                                    in1=redq[1][:], op=Alu.add)
            nc.vector.tensor_tensor(out=redq[2][:], in0=redq[2][:],
                                    in1=redq[3][:], op=Alu.add)
            nc.vector.tensor_tensor(out=pooled[:], in0=redq[0][:],
                                    in1=redq[2][:], op=Alu.add)
            nc.vector.tensor_tensor(out=pooled[:], in0=pooled[:],
                                    in1=rsq_t[:], op=Alu.mult)

            # ---- transpose pooled -> x_enc [64, 1024] ----
            x_enc = cpool.tile([64, 1024], f32, tag="xenc")
            for bq in range(4):
                tp = ppool.tile([128, 128], f32, tag="tp", space="PSUM")
                nc.tensor.transpose(out=tp[:], in_=pooled[:, bq * 128:(bq + 1) * 128],
                                    identity=eye_t[:])
                nc.vector.tensor_copy(out=x_enc[:, 256 * bq:256 * bq + 128],
                                      in_=tp[0:64, :])
                nc.vector.tensor_copy(out=x_enc[:, 256 * bq + 128:256 * (bq + 1)],
                                      in_=tp[64:128, :])

            # ---- encoder MLP ----
            h1 = cpool.tile([128, 1024], f32, tag="h1")
            h2 = cpool.tile([128, 1024], f32, tag="h2")
            for mi, htile in ((0, h1), (1, h2)):
                for half in range(2):
                    ps = ppool.tile([128, 512], f32, tag="encps", space="PSUM")
                    nc.tensor.matmul(out=ps[:],
                                     lhsT=we1_t[:, mi * 128:(mi + 1) * 128],
                                     rhs=x_enc[:, half * 512:(half + 1) * 512],
                                     start=True, stop=True)
                    nc.scalar.activation(out=htile[:, half * 512:(half + 1) * 512],
                                         in_=ps[:], func=Act.Relu,
                                         bias=b1_t[:, mi:mi + 1], scale=1.0)
            z_un = cpool.tile([64, 1024], f32, tag="zun")
            for half in range(2):
                ps = ppool.tile([64, 512], f32, tag="mups", space="PSUM")
                nc.tensor.matmul(out=ps[:], lhsT=w2a_t[:],
                                 rhs=h1[:, half * 512:(half + 1) * 512],
                                 start=True, stop=False)
                nc.tensor.matmul(out=ps[:], lhsT=w2b_t[:],
                                 rhs=h2[:, half * 512:(half + 1) * 512],
                                 start=False, stop=True)
                nc.scalar.activation(out=z_un[:, half * 512:(half + 1) * 512],
                                     in_=ps[:], func=Act.Identity,
                                     bias=b2_t[:], scale=1.0)
            z_st = cpool.tile([128, 512], f32, tag="zst")
            nc.sync.dma_start(out=z_st[0:64, :], in_=z_un[:, 0:512])
            nc.sync.dma_start(out=z_st[64:128, :], in_=z_un[:, 512:1024])

            # free phase-1 PSUM/gather space before the loop pools open
            phase_ctx.close()
            ppool = phase_ctx.enter_context(
                tc.tile_pool(name="psum_loop", bufs=2, space="PSUM"))

            # ---- phase 2: diffusion loop ----
            w_cur = spool.tile([128, 512], f32, tag="w")
            nc.sync.dma_start(out=w_cur[:], in_=w0_d[:])

            for t in range(T - 1, -1, -1):
                silu_scale = float(1.0 / (beta[t] * sr[t]))
                clip_b = float(beta[t])
                stt_s = float(stt_scalar[t])

                slab = npool.tile([128, 768], sn_dt, tag="sn")
                nc.sync.dma_start(out=slab[:], in_=sn_d[t])
                wcs = slab[:, 512:640]
                wos = slab[:, 640:768]

                w_nxt = spool.tile([128, 512], f32, tag="w")
                for s0 in range(2):
                    sl = slice(s0 * SUB, (s0 + 1) * SUB)
                    ps1 = ppool.tile([128, SUB], f32, tag=f"ps1_{s0}",
                                     space="PSUM")
                    nc.tensor.matmul(out=ps1[:], lhsT=win_t[:],
                                     rhs=w_cur[:, sl], start=True, stop=False)
                    nc.tensor.matmul(out=ps1[:], lhsT=wcs,
                                     rhs=z_st[:, sl], start=False, stop=True)
                    hh = wpool.tile([128, SUB], f32, tag=f"hh_{s0}")
                    nc.scalar.activation(out=hh[:], in_=ps1[:], func=Act.Silu,
                                         bias=tb_t[:, t:t + 1], scale=silu_scale)
                    ps2 = ppool.tile([128, SUB], f32, tag=f"ps2_{s0}",
                                     space="PSUM")
                    nc.tensor.matmul(out=ps2[:], lhsT=eye_t[:],
                                     rhs=w_cur[:, sl], start=True, stop=False)
                    nc.tensor.matmul(out=ps2[:], lhsT=wos,
                                     rhs=hh[:], start=False, stop=True)
                    x0 = wpool.tile([128, SUB], f32, tag=f"x0_{s0}")
                    nc.vector.tensor_scalar(out=x0[:], in0=ps2[:],
                                            scalar1=clip_b, scalar2=-clip_b,
                                            op0=Alu.min, op1=Alu.max)
                    if t > 0:
                        tmp = wpool.tile([128, SUB], f32, tag=f"tmp_{s0}")
                        nc.gpsimd.tensor_tensor(out=tmp[:], in0=w_cur[:, sl],
                                                in1=slab[:, sl], op=Alu.add)
                        in1 = tmp[:]
                    else:
                        in1 = slab[:, sl]
                    nc.vector.scalar_tensor_tensor(
                        out=w_nxt[:, sl], in0=x0[:], scalar=stt_s,
                        in1=in1, op0=Alu.mult, op1=Alu.add)
                w_cur = w_nxt

            nc.sync.dma_start(out=out_d[:], in_=w_cur[:])

    nc.compile()
    return nc


def _get_program(sn_dtype_np, K):
    key = ("prog", np.dtype(sn_dtype_np).name, K)
    if key not in _cache:
        _cache[key] = _build_program(sn_dtype_np, K)
    return _cache[key]


# ----------------------------------------------------------------------------
# kernel entry
# ----------------------------------------------------------------------------

SN_DTYPE = np.float32


def _pack_gather_indices(item_seq):
    """Quarter-split + pad the lookup indices for dma_gather.

    Returns (K, idx_arrays[NCORES]) with idx[c] int16 [128, NCALL*NIDX/16].
    """
    q_of = item_seq // QROWS                   # [B, L]
    loc = (item_seq - q_of * QROWS).astype(np.int64)
    counts = np.zeros((B, NQ), np.int64)
    for q in range(NQ):
        counts[:, q] = (q_of == q).sum(axis=1)
    maxc = int(counts.max())
    K = next((k for k in K_LADDER if k >= maxc), None)
    if K is None:
        raise RuntimeError(f"gather slot overflow: max count {maxc}")
    # slots[r, q, k] = local index (pad = QROWS -> appended zero row)
    slots = np.full((B, NQ, K), QROWS, np.int64)
    rank = (np.cumsum(np.eye(NQ, dtype=np.int64)[q_of], axis=1) - 1)  # [B,L,NQ]
    rank_l = np.take_along_axis(rank, q_of[:, :, None], axis=2)[:, :, 0]
    rr = np.repeat(np.arange(B), L)
    slots[rr, q_of.ravel(), rank_l.ravel()] = loc.ravel()

    idx_arrays = []
    ncall = (NJ // 2) * NQ
    nidx = 128 * 2 * K
    for c in range(NCORES):
        sl = slots[c * BS:(c + 1) * BS]        # [1024, NQ, K]
        sv = sl.reshape(NJ, 128, NQ, K)        # row = 128*j + p
        out = np.empty((ncall, 128, nidx // 16), np.int16)
        for jp in range(NJ // 2):
            for q in range(NQ):
                ci = jp * NQ + q
                sub = sv[2 * jp:2 * jp + 2, :, q, :]       # [jj, p, K]
                flat = sub.transpose(0, 2, 1).reshape(-1)  # i=(jj*K+k)*128+p
                wrap = flat.reshape(nidx // 16, 16).T.astype(np.int16)
                out[ci] = np.tile(wrap, (8, 1))
        idx_arrays.append(np.ascontiguousarray(
            out.transpose(1, 0, 2).reshape(128, ncall * (nidx // 16))))
    return K, idx_arrays


def _prep_inputs(inputs):
    """Build the per-core in_maps (host-side marshaling only)."""
    item_seq = np.asarray(inputs["item_seq"]).astype(np.int64)
    item_emb = np.ascontiguousarray(np.asarray(inputs["item_emb"], np.float32))
    W_enc1 = np.asarray(inputs["W_enc1"], np.float32)
    b_enc1 = np.asarray(inputs["b_enc1"], np.float32)
    W_enc2 = np.asarray(inputs["W_enc2"], np.float32)
    b_enc2 = np.asarray(inputs["b_enc2"], np.float32)
    W_in = np.asarray(inputs["W_in"], np.float32)
    b_in = np.asarray(inputs["b_in"], np.float32)
    W_t = np.asarray(inputs["W_t"], np.float32)
    b_tv = np.asarray(inputs["b_t"], np.float32)
    W_c = np.asarray(inputs["W_c"], np.float32)
    b_c = np.asarray(inputs["b_c"], np.float32)
    W_out = np.asarray(inputs["W_out"], np.float32)
    b_out = np.asarray(inputs["b_out"], np.float32)

    con = _derived_consts()
    sr, srm1 = con["sr"], con["srm1"]
    a_t, b_t = con["a_t"], con["b_t"]
    noise_coef, srm1_prev = con["noise_coef"], con["srm1_prev"]
    beta, beta_prev = con["beta"], con["beta_prev"]

    # per-partition bias table [128, T]
    half = D // 2
    freqs = np.exp(-np.log(10000.0) * np.arange(half, dtype=np.float32) / half)
    boWin = (b_out @ W_in).astype(np.float64)
    tbias = np.zeros((D, T), np.float32)
    for t in range(T):
        ang = np.float32(t) * freqs
        temb = np.concatenate([np.cos(ang), np.sin(ang)])
        tbias[:, t] = (temb @ W_t + b_tv + b_in + b_c
                       + (srm1[t] / sr[t]) * boWin).astype(np.float32)
    tbias_st = np.concatenate([tbias, tbias], axis=0)      # [128, T]

    x_init, noise = _host_noise()

    # noise slab [T, B, D]: beta_prev * (noise_coef*n + bracket)
    bracket = ((b_t * srm1 - srm1_prev)[:, None] * b_out[None, :])
    sn = noise * (beta_prev * noise_coef)[:, None, None].astype(np.float32)
    sn += (beta_prev[:, None] * bracket).astype(np.float32)[:, None, :]
    sn = sn.astype(SN_DTYPE)

    # per-step scaled weight blockdiags [T, 128, 256]
    wc_bd = _blockdiag(W_c)
    wout_bd = _blockdiag(W_out)
    wslab = np.empty((T, 128, 256), SN_DTYPE)
    for t in range(T):
        wslab[t, :, 0:128] = (beta[t] * sr[t]) * wc_bd
        wslab[t, :, 128:256] = (-beta[t] * srm1[t]) * wout_bd

    w0 = (beta[T - 1] * (sr[T - 1] * x_init - srm1[T - 1] * b_out)).astype(np.float32)

    cnt = (item_seq != 0).sum(1).astype(np.float32)
    with np.errstate(divide="ignore"):
        rsq = (1.0 / np.sqrt(cnt)).astype(np.float32)

    K, idx_arrays = _pack_gather_indices(item_seq)

    # quarter tables with an appended zero row (padding target)
    embq = {}
    for q in range(NQ):
        t = np.zeros((QROWS + 1, D), np.float32)
        t[:QROWS] = item_emb[q * QROWS:(q + 1) * QROWS]
        embq[f"embq{q}"] = t

    # shared (replicated) tensors
    shared = {
        **embq,
        "wenc1": W_enc1,
        "w2a": np.ascontiguousarray(W_enc2[0:128, 0:64]),
        "w2b": np.ascontiguousarray(W_enc2[128:256, 0:64]),
        "b1": np.ascontiguousarray(b_enc1.reshape(2, 128).T),
        "b2": np.ascontiguousarray(b_enc2[0:64].reshape(64, 1)),
        "win_bd": _blockdiag(W_in),
        "eye128": np.eye(128, dtype=np.float32),
        "tbias": tbias_st,
    }

    in_maps = []
    for c in range(NCORES):
        rows = slice(c * BS, (c + 1) * BS)
        idx_c = idx_arrays[c]
        rsq_c = rsq[rows].reshape(NJ, 128).T                # [128, 8]
        rsq_full = np.ascontiguousarray(
            np.repeat(rsq_c[:, :, None], D, axis=2).reshape(128, 512))
        sn_c = np.empty((T, 128, 768), SN_DTYPE)
        sn_c[:, :, 0:512] = (sn[:, rows].reshape(T, 2, 512, D)
                             .transpose(0, 1, 3, 2).reshape(T, 128, 512))
        sn_c[:, :, 512:768] = wslab
        m = dict(shared)
        m["idx"] = idx_c
        m["rsq"] = rsq_full
        m["w0"] = _stack(w0[rows])
        m["sn"] = sn_c
        in_maps.append(m)
    return K, in_maps


def kernel(**inputs):
    from concourse.bass_utils import run_bass_kernel_spmd

    K, in_maps = _prep_inputs(inputs)
    nc = _get_program(SN_DTYPE, K)
    res = run_bass_kernel_spmd(nc, in_maps, list(range(NCORES)),
                               trace=bool(int(os.environ.get("KTRACE", "0"))))
    _cache["last_result"] = res
    out = np.empty((B, D), np.float32)
    for c in range(NCORES):
        out[c * BS:(c + 1) * BS] = _unstack(res.results[c]["out"])
    return out
